# revision 2
# baseline (speedup 1.0000x reference)
"""Trainium2 Bass kernel v4 for DigitConvolutionalModel (dense_cnn).

Model: x[B,784] -> 3x3 valid conv on 28x28 -> flatten(676) -> fc1(128)+relu
       -> fc2(10).

Strategy:
  * Conv folded into fc1 on host: W_eff[128,784]. Pure data parallel:
    batch 65536 -> 8 cores x 8192 rows.
  * x shipped per-core transposed [784, 8192] in fp8 e3m4 (halves HBM
    traffic); weights fp16 (mixed-dtype matmul, exact on HW). Rel err
    ~0.95% vs the 2e-2 gate.
  * K=784 = 6x128 chunks + 16 tail. Accumulation groups run tail-LAST so
    the PE starts on chunk 0 before tail operands land. Last supergroup
    runs tail-first to shorten the end chain.
  * Tail matmuls (K=16) packed 4-at-a-time into distinct PE row groups;
    fc2 matmuls (M=10) packed 4-at-a-time into distinct col groups writing
    one PSUM tile at partition strips 32j..32j+9. fc2(s) is scheduled
    after chunks(s+1) so it never waits on the activation latency.
  * All weights in ONE [128,906] fp16 DMA (w1 c-major | tail strips | w2);
    one bias DMA; one compact pre-stripped x-tail DMA (all ACT HWDGE).
  * x: supergroup 0 split per-chunk (c-outer matmuls, DMA-paced start);
    s>=1 split in column halves [128, 6*1024] (j-outer per half).
  * Out: [106,512] fp16 strips per supergroup on the ACT ring; host
    re-gathers.
"""

import numpy as np

B_FULL = 65536
N_CORES = 8
B_LOC = B_FULL // N_CORES  # 8192
KF = 784
P = 128
KCH = KF // P  # 6 full K chunks
KT = KF - KCH * P  # 16 tail rows
H = 128
O = 10
NT = 512  # tile width (PSUM fp32 bank)
SG = 2048  # supergroup width (4 tiles)
NSG = B_LOC // SG  # 4 supergroups
HW_ = SG // 2  # column half width (1024)
WCOLS = KCH * H + H + O  # 906

_CACHED = {}


def _build_nc(b_loc=B_LOC, repeat=1, xbufs=4, ps1_bufs=6, ps2_bufs=2,
              h_bufs=9, bench=False, warmup_mms=7):
    import concourse.bacc as bacc
    import concourse.tile as tile
    from concourse import mybir

    f32 = mybir.dt.float32
    f16 = mybir.dt.float16
    f8 = mybir.dt.float8e3
    AF = mybir.ActivationFunctionType

    nsg = b_loc // SG
    jpg = SG // NT  # tiles per supergroup = 4

    nc = bacc.Bacc(
        "TRN2", target_bir_lowering=False, debug=False, enable_asserts=False,
        enable_partition_id=False,
    )
    xT = nc.dram_tensor("xT", [KF, b_loc], f8, kind="ExternalInput")
    wall = nc.dram_tensor("wall", [P, WCOLS], f16, kind="ExternalInput")
    xt4 = nc.dram_tensor("xt4", [112, b_loc // 4], f8, kind="ExternalInput")
    bias = nc.dram_tensor("bias", [P, 2], f32, kind="ExternalInput")
    # bench mode: all reps alias the same out region (identical values) so
    # the PJRT download size stays constant across repeat counts
    orep = 1 if bench else repeat
    outS = nc.dram_tensor("outS", [106, orep * nsg * NT], f16,
                          kind="ExternalOutput")

    with tile.TileContext(nc) as tc:
        with (
            tc.tile_pool(name="wpool", bufs=1) as wpool,
            tc.tile_pool(name="xpool", bufs=xbufs) as xpool,
            tc.tile_pool(name="hpool", bufs=h_bufs) as hpool,
            tc.tile_pool(name="opool", bufs=2) as opool,
            tc.tile_pool(name="ps1", bufs=ps1_bufs, space="PSUM") as ps1,
            tc.tile_pool(name="ps2", bufs=ps2_bufs, space="PSUM") as ps2,
        ):
            # --- weights on ACT HWDGE ring (SP ring stays x-only) ---
            ws = wpool.tile([P, WCOLS], f16)
            nc.scalar.dma_start(ws[:], wall[:])
            w1 = ws[:, 0 : KCH * H]
            w1tail = ws[0:112, KCH * H : KCH * H + H]
            w2 = ws[:, KCH * H + H : WCOLS]
            xtail = wpool.tile([112, b_loc // 4], f8)
            bs = wpool.tile([P, 2], f32)
            b1s = bs[:, 0:1]
            b2strip = bs[0:106, 1:2]

            def load_tail_bias():
                # issued after the first x chunks: first matmul needs only
                # ws + chunk 0, so these can trail
                nc.scalar.dma_start(xtail[:], xt4[:])
                nc.scalar.dma_start(bs[:], bias[:])

            def load_xg(rep, s):
                n0 = s * SG
                t = xpool.tile([P, KCH * SG], f8, name=f"xg{rep}_{s}",
                               tag="xg")
                if rep == 0 and s == 0:
                    # layout A: c-major full width [c, n] (2048 cols)
                    for c in range(KCH):
                        nc.sync.dma_start(
                            t[:, c * SG : (c + 1) * SG],
                            xT[c * P : (c + 1) * P, n0 : n0 + SG])
                else:
                    # layout B: halves [h][c, n] (1024 cols each)
                    for hh in range(2):
                        nh = n0 + hh * HW_
                        nc.sync.dma_start(
                            t[:, hh * KCH * HW_ : (hh + 1) * KCH * HW_]
                            .rearrange("p (c n) -> p c n", c=KCH),
                            xT[0 : KCH * P, nh : nh + HW_].rearrange(
                                "(c p) n -> p c n", p=P))
                return t

            def xcol(xg, layout, j, c):
                # columns [j*NT, (j+1)*NT) of chunk c
                if layout == "A":
                    return xg[:, c * SG + j * NT : c * SG + (j + 1) * NT]
                hh, jj = divmod(j, 2)
                off = hh * KCH * HW_ + c * HW_ + jj * NT
                return xg[:, off : off + NT]

            def chunks_tile(psum, xg, layout, j, tail_first):
                for c in range(KCH):
                    nc.tensor.matmul(
                        psum[:], w1[:, c * H : (c + 1) * H],
                        xcol(xg, layout, j, c),
                        start=(not tail_first and c == 0),
                        stop=(tail_first and c == KCH - 1))

            def tail_mm(psums, s, j, tail_first):
                nc.tensor.matmul(
                    psums[j][:],
                    w1tail[32 * j : 32 * j + KT, :],
                    xtail[32 * j : 32 * j + KT, s * NT : (s + 1) * NT],
                    start=tail_first, stop=not tail_first,
                    tile_position=(32 * j, 0))

            def fc2_batch(rep, s, hs):
                psum2 = ps2.tile([P, NT], f32, name=f"ps2_{rep}_{s}",
                                 tag="ps2")
                for j in range(jpg):
                    nc.tensor.matmul(
                        psum2[32 * j : 32 * j + O, :], w2, hs[j][:],
                        start=True, stop=True, tile_position=(0, 32 * j))
                o = opool.tile([106, NT], f16, name=f"o_{rep}_{s}", tag="o")
                nc.vector.tensor_scalar_add(o[:], psum2[0:106, :], b2strip)
                co = ((rep if not bench else 0) * nsg + s) * NT
                nc.scalar.dma_start(outS[:, co : co + NT], o[:])

            # PE pre-warm: dummy matmuls on never-written scratch SBUF keep
            # the PE busy during the initial DMA wait so the HAM clock gate
            # reaches 8/8 before the first real matmul (values never read)
            if warmup_mms:
                scratch = wpool.tile([P, NT], f16, name="warm_scratch")
                nc.vector.memset(scratch[:], 0.0)
                wpsum = ps2.tile([P, NT], f32, name="warm_ps", tag="ps2")
                for i in range(warmup_mms):
                    nc.tensor.matmul(wpsum[:], scratch[:, 0:P], scratch[:],
                                     start=True, stop=True)

            pending = None  # (rep, s, hs) awaiting fc2
            xg_next = load_xg(0, 0)
            load_tail_bias()
            for rep in range(repeat):
                for s in range(nsg):
                    xg = xg_next
                    layout = "A" if (rep == 0 and s == 0) else "B"
                    last = rep == repeat - 1 and s == nsg - 1
                    nrep, ns = (rep, s + 1) if s + 1 < nsg else (rep + 1, 0)
                    if nrep < repeat:
                        xg_next = load_xg(nrep, ns)
                    psums = [ps1.tile([P, NT], f32, name=f"ps_{rep}_{s}_{j}",
                                      tag="ps") for j in range(jpg)]
                    hs = []
                    if last:
                        # tail-first: shortens the end chain
                        for j in range(jpg):
                            tail_mm(psums, s, j, tail_first=True)
                        # flush the deferred fc2 before the final chunk run
                        if pending is not None:
                            fc2_batch(pending[0], pending[1], pending[2])
                            pending = None
                        for j in range(jpg):
                            chunks_tile(psums[j], xg, layout, j,
                                        tail_first=True)
                            h = hpool.tile([P, NT], f16,
                                           name=f"h_{rep}_{s}_{j}", tag="h")
                            nc.scalar.activation(h[:], psums[j][:], AF.Relu,
                                                 bias=b1s, scale=1.0)
                            hs.append(h)
                    else:
                        if layout == "A":
                            # c-outer: paced by per-chunk split DMAs
                            for c in range(KCH):
                                for j in range(jpg):
                                    nc.tensor.matmul(
                                        psums[j][:],
                                        w1[:, c * H : (c + 1) * H],
                                        xcol(xg, layout, j, c),
                                        start=(c == 0), stop=False)
                        else:
                            for j in range(jpg):
                                chunks_tile(psums[j], xg, layout, j,
                                            tail_first=False)
                        # tails: 4 concurrent row-group matmuls close groups
                        for j in range(jpg):
                            tail_mm(psums, s, j, tail_first=False)
                        for j in range(jpg):
                            h = hpool.tile([P, NT], f16,
                                           name=f"h_{rep}_{s}_{j}", tag="h")
                            nc.scalar.activation(h[:], psums[j][:], AF.Relu,
                                                 bias=b1s, scale=1.0)
                            hs.append(h)
                    # fc2 for the previous supergroup: its activations are
                    # long done, so the PE never waits on them
                    if pending is not None:
                        fc2_batch(pending[0], pending[1], pending[2])
                    pending = (rep, s, hs)
                    if last:
                        fc2_batch(rep, s, hs)
                        pending = None
    nc.compile()
    return nc


def _get_nc():
    if "nc" not in _CACHED:
        _CACHED["nc"] = _build_nc()
    return _CACHED["nc"]


def _prep_weights(conv_w, fc1_w, fc1_b, fc2_w, fc2_b):
    w1 = np.asarray(fc1_w, np.float64).reshape(H, 26, 26)
    cw = np.asarray(conv_w, np.float64)
    w_eff = np.zeros((H, 28, 28), np.float64)
    for di in range(3):
        for dj in range(3):
            w_eff[:, di : di + 26, dj : dj + 26] += cw[di, dj] * w1
    w1t = np.ascontiguousarray(w_eff.reshape(H, KF).T.astype(np.float16))
    # wall[p, c*H+m] = w1t[c*128+p, m]; cols 768:896 tail strips; 896:906 w2
    wall = np.zeros((P, WCOLS), np.float16)
    wall[:, 0 : KCH * H] = (
        w1t[0 : KCH * P].reshape(KCH, P, H).transpose(1, 0, 2).reshape(P, KCH * H))
    for g in range(4):
        wall[32 * g : 32 * g + KT, KCH * H : KCH * H + H] = w1t[KCH * P : KF, :]
    wall[:, KCH * H + H :] = np.asarray(fc2_w, np.float32).T.astype(np.float16)
    bias = np.zeros((P, 2), np.float32)
    bias[:, 0] = np.asarray(fc1_b, np.float32)
    for g in range(4):
        bias[32 * g : 32 * g + O, 1] = np.asarray(fc2_b, np.float32)
    return wall, bias


def _prep_xtail4(xTc):
    # strip g cols [512s:512s+512] = xT[768:784, 2048s + 512g : +512]
    xt4 = np.zeros((112, B_LOC // 4), xTc.dtype)
    tail = xTc[KCH * P : KF]  # [16, 8192]
    for g in range(4):
        for s in range(NSG):
            xt4[32 * g : 32 * g + KT, NT * s : NT * (s + 1)] = (
                tail[:, SG * s + NT * g : SG * s + NT * (g + 1)])
    return xt4


def _make_in_maps(x, conv_w, fc1_w, fc1_b, fc2_w, fc2_b):
    import ml_dtypes

    x = np.asarray(x, np.float32)
    assert x.shape == (B_FULL, KF), x.shape
    wall, bias = _prep_weights(conv_w, fc1_w, fc1_b, fc2_w, fc2_b)

    x8 = x.astype(ml_dtypes.float8_e3m4)
    in_maps = []
    for c in range(N_CORES):
        xT = np.ascontiguousarray(x8[c * B_LOC : (c + 1) * B_LOC].T)
        in_maps.append({"xT": xT, "wall": wall, "xt4": _prep_xtail4(xT),
                        "bias": bias})
    return in_maps


def kernel(x, conv_w, fc1_w, fc1_b, fc2_w, fc2_b, _trace=False):
    from concourse.bass_utils import run_bass_kernel_spmd

    in_maps = _make_in_maps(x, conv_w, fc1_w, fc1_b, fc2_w, fc2_b)

    nc = _get_nc()
    try:
        res = run_bass_kernel_spmd(
            nc, in_maps, core_ids=list(range(N_CORES)), trace=_trace
        )
    except ModuleNotFoundError:
        import os

        os.environ["BASS_NEVER_TRACE"] = "1"
        res = run_bass_kernel_spmd(
            nc, in_maps, core_ids=list(range(N_CORES)), trace=False
        )
    out = np.empty((B_FULL, O), np.float32)
    for c in range(N_CORES):
        outS = res.results[c]["outS"].astype(np.float32)  # [106, NSG*NT]
        loc = out[c * B_LOC : (c + 1) * B_LOC]
        for s in range(NSG):
            for g in range(4):
                blk = outS[32 * g : 32 * g + O, s * NT : (s + 1) * NT]
                loc[s * SG + g * NT : s * SG + (g + 1) * NT, :] = blk.T
    if _trace:
        _CACHED["last_results"] = res
    return out

